# revision 13
# baseline (speedup 1.0000x reference)
"""Trainium2 8-core kernel for a GPT-style transformer block.

Strategy (v2):
  - Token-parallel everywhere except attention: each core owns a contiguous
    512-token span (core i -> batch i//4, span i%4). LayerNorms, QKV, proj,
    MLP and residuals are purely local to the core's tokens.
  - Attention is head-parallel: ONE fused AllToAll redistributes Q^T/K^T
    (feature-major) and V (token-major) so core j holds head-pair j for all
    4096 tokens. A tiny warmup AllGather is triggered at t~0 to absorb the
    ~100us one-time collective-establishment cost during QKV compute.
  - All weights are pre-laid-out on the host into SBUF-image format
    [128, n*cols] so every load is one contiguous 2D DMA (the per-DMA issue
    cost on the sequencers is ~0.6us, so DMA count matters).
  - Softmax denominators come free via ones-columns embedded in V; the
    normalize step uses vector.reciprocal (keeps the Scalar engine's
    activation table on Exp for the whole attention phase).
  - LN2 statistics accumulate inside the proj loop; residual uses bf16 x.
"""

import sys

sys.path.insert(0, "/opt/trn_rl_repo")

import numpy as np
import ml_dtypes

import concourse.bass as bass
import concourse.mybir as mybir
import concourse.tile as tile
from concourse import bacc, bass_utils

BF16 = mybir.dt.bfloat16
F32 = mybir.dt.float32
AF = mybir.ActivationFunctionType
ALU = mybir.AluOpType
NP_BF16 = ml_dtypes.bfloat16

B, T, C, H, HS, FF = 2, 2048, 1024, 16, 64, 4096
CORES = 8
S = 512            # tokens per core
NCT = C // 128     # 8 feature tiles
NFT = FF // 128    # 32 mlp hidden tiles
NTT = S // 128     # 4 token tiles per core
QCH = 256          # query chunk
NQC = T // QCH     # 8 query chunks per batch
NKT = T // 128     # 16 key tiles per batch
EPS = 1e-5
QBLK = 65536       # per-block elems of q (and k, and v) in fused A2A


def build(flags):
    (use_bq, use_bk, use_bv, use_bproj, use_bfc, use_bmlp,
     use_ln1wb, use_ln2wb, debug) = flags

    nc = bacc.Bacc("TRN2", target_bir_lowering=False, debug=False,
                   num_devices=CORES)

    # ---------------- DRAM parameters (host-side SBUF images) ----------
    xt = nc.dram_tensor("xt", [128, NCT * S], BF16, kind="ExternalInput")
    awq = nc.dram_tensor("awq", [128, NCT * C], BF16, kind="ExternalInput")
    awk = nc.dram_tensor("awk", [128, NCT * C], BF16, kind="ExternalInput")
    awv = nc.dram_tensor("awv", [128, NCT * C], BF16, kind="ExternalInput")
    pw = nc.dram_tensor("pw", [128, NCT * C], BF16, kind="ExternalInput")
    fcw = nc.dram_tensor("fcw", [128, NCT * FF], BF16, kind="ExternalInput")
    mlpw = nc.dram_tensor("mlpw", [128, NFT * C], BF16, kind="ExternalInput")
    masks = nc.dram_tensor("masks", [128, 3 * QCH * 4], BF16,
                           kind="ExternalInput")
    b_q = nc.dram_tensor("b_q", [128, NCT], F32, kind="ExternalInput")
    b_k = nc.dram_tensor("b_k", [128, NCT], F32, kind="ExternalInput")
    b_v = nc.dram_tensor("b_v", [1, C], BF16, kind="ExternalInput")
    b_proj = nc.dram_tensor("b_proj", [128, NCT], F32, kind="ExternalInput")
    b_fc = nc.dram_tensor("b_fc", [128, NFT], F32, kind="ExternalInput")
    b_mlp = nc.dram_tensor("b_mlp", [128, NCT], F32, kind="ExternalInput")
    ln1w_d = nc.dram_tensor("ln1w", [128, NCT], F32, kind="ExternalInput")
    ln1b_d = nc.dram_tensor("ln1b", [128, NCT], F32, kind="ExternalInput")
    ln2w_d = nc.dram_tensor("ln2w", [128, NCT], F32, kind="ExternalInput")
    ln2b_d = nc.dram_tensor("ln2b", [128, NCT], F32, kind="ExternalInput")
    out_d = nc.dram_tensor("out", [C, S], F32, kind="ExternalOutput")

    with tile.TileContext(nc) as tc:
        _build_body(nc, tc, locals(), flags)
    nc.compile()
    return nc


def _build_body(nc, tc, t_, flags):
    (use_bq, use_bk, use_bv, use_bproj, use_bfc, use_bmlp,
     use_ln1wb, use_ln2wb, debug) = flags
    xt, awq, awk, awv = t_["xt"], t_["awq"], t_["awk"], t_["awv"]
    pw, fcw, mlpw, masks = t_["pw"], t_["fcw"], t_["mlpw"], t_["masks"]
    b_q, b_k, b_v, b_proj, b_fc, b_mlp = (t_["b_q"], t_["b_k"], t_["b_v"],
                                          t_["b_proj"], t_["b_fc"], t_["b_mlp"])
    ln1w_d, ln1b_d, ln2w_d, ln2b_d = (t_["ln1w_d"], t_["ln1b_d"],
                                      t_["ln2w_d"], t_["ln2b_d"])
    out_d = t_["out_d"]

    from contextlib import ExitStack
    es = ExitStack()

    consts = es.enter_context(tc.tile_pool(name="consts", bufs=1))
    dram = es.enter_context(tc.tile_pool(name="dram", bufs=1, space="DRAM"))

    # ---- collective DRAM tiles ----
    cc0_in = dram.tile([1, 128], BF16, name="cc0_in")
    cc0_out = dram.tile([CORES, 128], BF16, name="cc0_out")
    ccq_in = dram.tile([CORES, QBLK], BF16, name="ccq_in")
    ccq_out = dram.tile([CORES, QBLK], BF16, name="ccq_out")
    cck_in = dram.tile([CORES, QBLK], BF16, name="cck_in")
    cck_out = dram.tile([CORES, QBLK], BF16, name="cck_out")
    ccv_in = dram.tile([CORES, QBLK], BF16, name="ccv_in")
    ccv_out = dram.tile([CORES, QBLK], BF16, name="ccv_out")
    ccy_in = dram.tile([CORES, 128, S], BF16, name="ccy_in")
    ccy_out = dram.tile([CORES, 128, S], BF16, name="ccy_out")

    # ---- warmup collective: FIRST sync DMA + FIRST gpsimd instruction ----
    masks_sb = consts.tile([128, 3 * QCH * 4], BF16, name="masks_sb")
    nc.sync.dma_start(out=cc0_in, in_=masks[0:1, 0:128])
    nc.gpsimd.collective_compute(
        "AllGather", ALU.bypass,
        replica_groups=[list(range(CORES))],
        ins=[cc0_in[:, :].opt()],
        outs=[cc0_out[:, :].opt()])

    # ---- constants ----
    nc.sync.dma_start(out=masks_sb, in_=masks[:, :])
    mask0 = masks_sb[:, 0:4 * QCH]
    mask1 = masks_sb[:, 4 * QCH:8 * QCH]
    mask0x = masks_sb[:, 8 * QCH:10 * QCH]
    mask1x = masks_sb[:, 10 * QCH:12 * QCH]
    ones_col = consts.tile([128, 1], BF16, name="ones_col")
    nc.vector.memset(ones_col, 1.0)
    ones_row = consts.tile([1, 128], BF16, name="ones_row")
    nc.vector.memset(ones_row, 1.0)
    eps_t = consts.tile([1, 1], F32, name="eps_t")
    nc.vector.memset(eps_t, EPS)

    def load_const(name, dram_t, shape, dtype=F32):
        t = consts.tile(shape, dtype, name=name)
        nc.sync.dma_start(out=t, in_=dram_t[:, :])
        return t

    bq_sb = load_const("bq_sb", b_q, [128, NCT]) if use_bq else None
    bk_sb = load_const("bk_sb", b_k, [128, NCT]) if use_bk else None
    bv_sb = load_const("bv_sb", b_v, [1, C], BF16) if use_bv else None
    bproj_sb = load_const("bproj_sb", b_proj, [128, NCT]) if use_bproj else None
    bfc_sb = load_const("bfc_sb", b_fc, [128, NFT]) if use_bfc else None
    bmlp_sb = load_const("bmlp_sb", b_mlp, [128, NCT]) if use_bmlp else None
    ln1w_sb = load_const("ln1w_sb", ln1w_d, [128, NCT]) if use_ln1wb else None
    ln1b_sb = load_const("ln1b_sb", ln1b_d, [128, NCT]) if use_ln1wb else None
    ln2w_sb = load_const("ln2w_sb", ln2w_d, [128, NCT]) if use_ln2wb else None
    ln2b_sb = load_const("ln2b_sb", ln2b_d, [128, NCT]) if use_ln2wb else None

    def bcast(pspool, tag, src_bf, n):
        """[1, n] bf16 row -> [128, n] f32 PSUM via rank-1 matmul."""
        ps = pspool.tile([128, 512], F32, name=f"{tag}_bc", tag="ps")
        nc.tensor.matmul(ps[:, :n], ones_row[:, :], src_bf[:, :n],
                         start=True, stop=True)
        return ps

    def ln_finish(tag, pool, pspool, s_ps, q_ps):
        """From accumulated sum/sumsq [1,S] PSUM -> (r_b, sh_b) [128,S] bf16:
        normalized = src*r_b + sh_b."""
        mu = pool.tile([1, S], F32, name=f"{tag}_mu")
        nc.scalar.mul(mu, s_ps[:, :], 1.0 / C)
        msq = pool.tile([1, S], F32, name=f"{tag}_msq")
        nc.scalar.mul(msq, q_ps[:, :], 1.0 / C)
        mu2 = pool.tile([1, S], F32, name=f"{tag}_mu2")
        nc.vector.tensor_mul(mu2, mu, mu)
        var = pool.tile([1, S], F32, name=f"{tag}_var")
        nc.vector.tensor_sub(var, msq, mu2)
        lnv = pool.tile([1, S], F32, name=f"{tag}_lnv")
        nc.scalar.activation(lnv, var, AF.Ln, bias=eps_t, scale=1.0)
        rstd = pool.tile([1, S], F32, name=f"{tag}_rstd")
        nc.scalar.activation(rstd, lnv, AF.Exp, scale=-0.5)
        rstd_bf = pool.tile([1, S], BF16, name=f"{tag}_rstd_bf")
        nc.vector.tensor_copy(rstd_bf, rstd)
        nmurs = pool.tile([1, S], F32, name=f"{tag}_nmurs")
        nc.vector.tensor_mul(nmurs, mu, rstd)
        nmurs_bf = pool.tile([1, S], BF16, name=f"{tag}_nmurs_bf")
        nc.scalar.mul(nmurs_bf, nmurs, -1.0)
        r_ps = bcast(pspool, f"{tag}_r", rstd_bf, S)
        sh_ps = bcast(pspool, f"{tag}_sh", nmurs_bf, S)
        r_b = pool.tile([128, S], BF16, name=f"{tag}_r_b")
        nc.scalar.copy(r_b, r_ps[:, :S])
        sh_b = pool.tile([128, S], BF16, name=f"{tag}_sh_b")
        nc.scalar.copy(sh_b, sh_ps[:, :S])
        return r_b, sh_b

    def ln_apply(tag, pool, c, src, r_b, sh_b, w_sb, b_sb, use_wb):
        tmp = pool.tile([128, S], BF16, name=f"{tag}_tmp_{c}",
                        tag=f"{tag}_tmp", bufs=3)
        nc.vector.tensor_mul(tmp, src, r_b)
        o = pool.tile([128, S], BF16, name=f"{tag}_o_{c}")
        if use_wb:
            nc.vector.tensor_add(tmp, tmp, sh_b)
            nc.vector.tensor_scalar(
                out=o, in0=tmp,
                scalar1=w_sb[:, c:c + 1], scalar2=b_sb[:, c:c + 1],
                op0=ALU.mult, op1=ALU.add)
        else:
            nc.vector.tensor_add(o, tmp, sh_b)
        return o

    # =========================================================
    # Phase 1+2: LN1 and QKV projections
    # =========================================================
    xp = es.enter_context(tc.tile_pool(name="xt_pool", bufs=1))
    ln1_pool = tc.tile_pool(name="ln1_pool", bufs=1)
    qkv_pool = tc.tile_pool(name="qkv_pool", bufs=1)
    psA_pool = tc.tile_pool(name="psA", bufs=6, space="PSUM")

    xt_big = xp.tile([128, NCT * S], BF16, name="xt_big")
    nc.sync.dma_start(out=xt_big, in_=xt[:, :])

    with ln1_pool as lp, qkv_pool as qp, psA_pool as psA:
        awq_sb = lp.tile([128, NCT * C], BF16, name="awq_sb")
        nc.sync.dma_start(out=awq_sb, in_=awq[:, :])
        awk_sb = lp.tile([128, NCT * C], BF16, name="awk_sb")
        nc.sync.dma_start(out=awk_sb, in_=awk[:, :])
        awv_sb = lp.tile([128, NCT * C], BF16, name="awv_sb")
        nc.sync.dma_start(out=awv_sb, in_=awv[:, :])

        # LN1 stats
        s_ps = psA.tile([1, 512], F32, name="ln1_sps", tag="st", bufs=2)
        q_ps = psA.tile([1, 512], F32, name="ln1_qps", tag="st", bufs=2)
        for c in range(NCT):
            src = xt_big[:, c * S:(c + 1) * S]
            sq = lp.tile([128, S], BF16, name=f"ln1_sq_{c}",
                         tag="ln1_sq", bufs=3)
            nc.vector.tensor_mul(sq, src, src)
            nc.tensor.matmul(s_ps[:, :], ones_col[:, :], src,
                             start=(c == 0), stop=(c == NCT - 1))
            nc.tensor.matmul(q_ps[:, :], ones_col[:, :], sq[:, :],
                             start=(c == 0), stop=(c == NCT - 1))
        r_b, sh_b = ln_finish("ln1", lp, psA, s_ps, q_ps)
        ln1t = [ln_apply("ln1", lp, c, xt_big[:, c * S:(c + 1) * S],
                         r_b, sh_b, ln1w_sb, ln1b_sb, use_ln1wb)
                for c in range(NCT)]

        # Q and K, feature-major [C, S], into one big output tile each;
        # each gets its own AllToAll triggered as soon as its data is out.
        for which, w_sb, bias_sb, useb, ccin, ccout in (
                ("q", awq_sb, bq_sb, use_bq, ccq_in, ccq_out),
                ("k", awk_sb, bk_sb, use_bk, cck_in, cck_out)):
            obig = qp.tile([128, NCT * S], BF16, name=f"{which}o_big")
            for hp in range(NCT):
                ps = psA.tile([128, 512], F32, name=f"{which}ps_{hp}", tag="ps")
                for c in range(NCT):
                    nc.tensor.matmul(
                        ps[:, :],
                        w_sb[:, c * C + hp * 128: c * C + (hp + 1) * 128],
                        ln1t[c][:, :],
                        start=(c == 0), stop=(c == NCT - 1))
                o = obig[:, hp * S:(hp + 1) * S]
                if useb:
                    nc.scalar.add(o, ps[:, :], bias_sb[:, hp:hp + 1])
                else:
                    nc.scalar.copy(o, ps[:, :])
            nc.sync.dma_start(
                out=ccin[:, :].rearrange("j (p s) -> p j s", p=128),
                in_=obig[:, :])
            nc.gpsimd.collective_compute(
                "AllToAll", ALU.bypass,
                replica_groups=[list(range(CORES))],
                ins=[ccin[:, :].opt()],
                outs=[ccout[:, :].opt()])

        # V, token-major [S, C]
        for tt in range(NTT):
            vo = qp.tile([128, C], BF16, name=f"vo_{tt}", tag="vo", bufs=2)
            for half in range(2):
                ps = psA.tile([128, 512], F32, name=f"vps_{tt}_{half}",
                              tag="ps")
                for c in range(NCT):
                    nc.tensor.matmul(
                        ps[:, :],
                        ln1t[c][:, tt * 128:(tt + 1) * 128],
                        awv_sb[:, c * C + half * 512: c * C + (half + 1) * 512],
                        start=(c == 0), stop=(c == NCT - 1 and not use_bv))
                if use_bv:
                    nc.tensor.matmul(
                        ps[:, :], ones_row[:, :],
                        bv_sb[:, half * 512:(half + 1) * 512],
                        start=False, stop=True)
                nc.scalar.copy(vo[:, half * 512:(half + 1) * 512], ps[:, :])
            nc.sync.dma_start(
                out=ccv_in[:, tt * (128 * 128):
                           (tt + 1) * (128 * 128)].rearrange(
                    "j (t f) -> t j f", f=128),
                in_=vo[:, :])

        nc.gpsimd.collective_compute(
            "AllToAll", ALU.bypass,
            replica_groups=[list(range(CORES))],
            ins=[ccv_in[:, :].opt()],
            outs=[ccv_out[:, :].opt()])

    # =========================================================
    # Weight prefetch (executes during collective establishment window)
    # =========================================================
    x2t_p = es.enter_context(tc.tile_pool(name="x2t_p", bufs=1))
    wt_pool = tc.tile_pool(name="wt_pool", bufs=1)
    wp = wt_pool.__enter__()
    pw_sb = wp.tile([128, NCT * C], BF16, name="pw_sb")
    nc.sync.dma_start(out=pw_sb, in_=pw[:, :])
    NFS = 4            # fc slabs
    FPS = NFT // NFS   # f-tiles per slab
    fc_view = fcw[:, :].rearrange("p (c f) -> p c f", f=FF)
    fw_sb = {}
    for sl in range(2):
        tl = wp.tile([128, NCT * FPS * 128], BF16, name=f"fw_{sl}",
                     tag="fw", bufs=2)
        nc.sync.dma_start(
            out=tl, in_=fc_view[:, :, sl * FPS * 128:(sl + 1) * FPS * 128])
        fw_sb[sl] = tl
    mlp_sb = {}
    for g in range(1):
        tl = wp.tile([128, 8 * C], BF16, name=f"mw_{g}", tag="mw", bufs=2)
        nc.sync.dma_start(out=tl, in_=mlpw[:, g * 8 * C:(g + 1) * 8 * C])
        mlp_sb[g] = tl

    # =========================================================
    # Phase 3: attention (my 2 heads, all tokens)
    # =========================================================
    att_pool = tc.tile_pool(name="att_pool", bufs=1)
    psB_pool = tc.tile_pool(name="psB", bufs=2, space="PSUM")
    with att_pool as ap, psB_pool as psB:
        qtb, ktb, vt = [], [], []
        for b in range(B):
            qt_t = ap.tile([128, T], BF16, name=f"qtb_{b}")
            nc.sync.dma_start(
                out=qt_t,
                in_=ccq_out[4 * b:4 * b + 4, :].rearrange(
                    "r (p s) -> p r s", p=128))
            kt_t = ap.tile([128, T], BF16, name=f"ktb_{b}")
            nc.sync.dma_start(
                out=kt_t,
                in_=cck_out[4 * b:4 * b + 4, :].rearrange(
                    "r (p s) -> p r s", p=128))
            qtb.append(qt_t)
            ktb.append(kt_t)
        for b in range(B):
            for kt in range(NKT):
                # per head: col 0 = ones (denominator), 1:64 zero, 64:128 = V
                v3 = ap.tile([128, 2, 128], BF16, name=f"vt_{b}_{kt}")
                r = 4 * b + kt // 4
                ro = (kt % 4) * 128
                nc.gpsimd.memset(v3[:, :, 0:1], 1.0)
                nc.gpsimd.memset(v3[:, :, 1:64], 0.0)
                nc.sync.dma_start(
                    out=v3[:, :, 64:128],
                    in_=ccv_out[r, ro * 128:(ro + 128) * 128].rearrange(
                        "(t h f) -> t h f", h=2, f=64))
                vt.append(v3)

        W2 = 2 * QCH
        for b in range(B):
            for p in reversed(range(NQC // 2)):
                qc = 2 * p
                qs = qc * QCH
                nsh = 2 * (qc + 1)          # shared key tiles
                y_A = psB.tile([128, W2], F32, name=f"yA_{b}_{p}", tag="ya",
                               bufs=4)
                y_B = psB.tile([128, W2], F32, name=f"yB_{b}_{p}", tag="ya",
                               bufs=4)
                ytA = ap.tile([128, W2], BF16, name=f"ytbA_{b}_{p}",
                              tag="ytA", bufs=3)
                ytB = ap.tile([128, W2], BF16, name=f"ytbB_{b}_{p}",
                              tag="ytB", bufs=3)
                # phase 1: all scores + exp (+mask) for this pair -- lets
                # score work run while the V AllToAll is still in flight
                es_AB = []
                for kt in range(nsh + 2):
                    shared = kt < nsh
                    ncols = W2 if shared else QCH
                    s_AB = psB.tile([128, 2 * W2], F32, name=f"s_{b}_{p}_{kt}",
                                    tag="ps2", bufs=2)
                    cols = slice(0, W2) if shared else slice(QCH, W2)
                    nc.tensor.matmul(s_AB[:, 0:ncols],
                                     ktb[b][0:64, kt * 128:(kt + 1) * 128],
                                     qtb[b][0:64, qs + cols.start:qs + W2],
                                     start=True, stop=True)
                    nc.tensor.matmul(s_AB[:, W2:W2 + ncols],
                                     ktb[b][64:128, kt * 128:(kt + 1) * 128],
                                     qtb[b][64:128, qs + cols.start:qs + W2],
                                     start=True, stop=True)
                    e_AB = ap.tile([128, 2 * W2], BF16, name=f"e_{b}_{p}_{kt}",
                                   tag="eAB", bufs=16)
                    if shared:
                        nc.scalar.activation(e_AB, s_AB[:, :], AF.Exp,
                                             scale=1.0 / np.sqrt(HS))
                        if kt == qc * 2:
                            nc.vector.tensor_mul(e_AB, e_AB, mask0)
                        elif kt == qc * 2 + 1:
                            nc.vector.tensor_mul(e_AB, e_AB, mask1)
                    else:
                        e3 = e_AB.rearrange("p (h q) -> p h q", h=2)
                        s3 = s_AB.rearrange("p (h q) -> p h q", h=2)
                        nc.scalar.activation(e3[:, :, 0:QCH], s3[:, :, 0:QCH],
                                             AF.Exp, scale=1.0 / np.sqrt(HS))
                        mx = mask0x if kt == nsh else mask1x
                        nc.vector.tensor_mul(
                            e_AB.rearrange("p (h q) -> p h q", h=2)[:, :, 0:QCH],
                            e_AB.rearrange("p (h q) -> p h q", h=2)[:, :, 0:QCH],
                            mx.rearrange("p (h q) -> p h q", h=2))
                    es_AB.append(e_AB)
                # phase 2: all AV accumulations
                for kt in range(nsh + 2):
                    shared = kt < nsh
                    cols = slice(0, W2) if shared else slice(QCH, W2)
                    ncols = W2 if shared else QCH
                    e_AB = es_AB[kt]
                    v3 = vt[b * NKT + kt]
                    nc.tensor.matmul(y_A[:, cols], v3[:, 0, :],
                                     e_AB[:, 0:ncols],
                                     start=(kt == 0), stop=(kt == nsh + 1),
                                     skip_group_check=True)
                    nc.tensor.matmul(y_B[:, cols], v3[:, 1, :],
                                     e_AB[:, W2:W2 + ncols],
                                     start=(kt == 0),
                                     stop=(kt == nsh + 1),
                                     skip_group_check=True)
                # normalize: den = row 0 of PSUM acc; recip on Vector engine
                for y_ps, ytb, hn in ((y_A, ytA, "A"), (y_B, ytB, "B")):
                    nc.vector.tensor_copy(ytb[64:128, :], y_ps[64:128, :])
                    rec = ap.tile([1, W2], F32, name=f"rec_{b}_{p}{hn}",
                                  tag="rec", bufs=4)
                    nc.vector.reciprocal_approx_fast(rec, y_ps[0:1, :])
                    rec_bf = ap.tile([1, W2], BF16, name=f"recbf_{b}_{p}{hn}",
                                     tag="recbf", bufs=4)
                    nc.vector.tensor_copy(rec_bf, rec)
                    den = ap.tile([128, W2], BF16, name=f"den_{b}_{p}{hn}",
                                  tag="den", bufs=4)
                    nc.gpsimd.partition_broadcast(den, rec_bf)
                    nc.vector.tensor_mul(ytb[64:128, :],
                                         ytb[64:128, :], den[64:128, :])
                j = 4 * b + p
                nc.sync.dma_start(out=ccy_in[j, 0:64, :], in_=ytA[64:128, :])
                nc.sync.dma_start(out=ccy_in[j, 64:128, :], in_=ytB[64:128, :])

        a2a_y = nc.gpsimd.collective_compute(
            "AllToAll", ALU.bypass,
            replica_groups=[list(range(CORES))],
            ins=[ccy_in[:, :, :].opt()],
            outs=[ccy_out[:, :, :].opt()])

    # =========================================================
    # Phase 4: proj + residual (+ LN2 stats inline)
    # =========================================================
    mlp_pool = tc.tile_pool(name="mlp_pool", bufs=1)
    psC_cm = tc.tile_pool(name="psC", bufs=6, space="PSUM")
    psC = psC_cm.__enter__()
    with mlp_pool as mp:
        yta = mp.tile([128, NCT * S], BF16, name="yta_big")
        nc.sync.dma_start(
            out=yta, in_=ccy_out[:, :, :].rearrange("j p s -> p j s"))
        s2_ps = psC.tile([1, 512], F32, name="ln2_sps", tag="st", bufs=2)
        q2_ps = psC.tile([1, 512], F32, name="ln2_qps", tag="st", bufs=2)
        x2t_sb, x2bf_sb = [], []
        for co in range(NCT):
            ps = psC.tile([128, 512], F32, name=f"prps_{co}", tag="ps")
            for ci in range(NCT):
                nc.tensor.matmul(ps[:, :],
                                 pw_sb[:, ci * C + co * 128:
                                       ci * C + (co + 1) * 128],
                                 yta[:, ci * S:(ci + 1) * S],
                                 start=(ci == 0), stop=(ci == NCT - 1))
            x2 = x2t_p.tile([128, S], F32, name=f"x2t_{co}")
            if use_bproj:
                nc.vector.scalar_tensor_tensor(
                    out=x2, in0=ps[:, :], scalar=bproj_sb[:, co:co + 1],
                    in1=xt_big[:, co * S:(co + 1) * S],
                    op0=ALU.add, op1=ALU.add)
            else:
                nc.vector.tensor_add(x2, ps[:, :],
                                     xt_big[:, co * S:(co + 1) * S])
            x2b = x2t_p.tile([128, S], BF16, name=f"x2bf_{co}")
            nc.vector.tensor_copy(x2b, x2)
            sq2 = mp.tile([128, S], BF16, name=f"ln2_sq_{co}",
                          tag="ln2_sq", bufs=3)
            nc.vector.tensor_mul(sq2, x2b, x2b)
            nc.tensor.matmul(s2_ps[:, :], ones_col[:, :], x2b[:, :],
                             start=(co == 0), stop=(co == NCT - 1))
            nc.tensor.matmul(q2_ps[:, :], ones_col[:, :], sq2[:, :],
                             start=(co == 0), stop=(co == NCT - 1))
            x2t_sb.append(x2)
            x2bf_sb.append(x2b)

        # Phase 5: LN2 finish + apply
        r2_b, sh2_b = ln_finish("ln2", mp, psC, s2_ps, q2_ps)
        ln2t = [ln_apply("ln2", mp, c, x2bf_sb[c], r2_b, sh2_b,
                         ln2w_sb, ln2b_sb, use_ln2wb)
                for c in range(NCT)]

        # Phase 6: fc + GELU  (fc weights in 4 slabs, ring of 2)
        for sl in range(2, NFS):
            tl = wp.tile([128, NCT * FPS * 128], BF16, name=f"fw_{sl}",
                         tag="fw", bufs=2)
            nc.sync.dma_start(
                out=tl,
                in_=fc_view[:, :, sl * FPS * 128:(sl + 1) * FPS * 128])
            fw_sb[sl] = tl
        ht = []
        for f in range(NFT):
            sl, fo = f // FPS, f % FPS
            ps = psC.tile([128, 512], F32, name=f"fcps_{f}", tag="ps")
            for c in range(NCT):
                nc.tensor.matmul(
                    ps[:, :],
                    fw_sb[sl][:, c * FPS * 128 + fo * 128:
                              c * FPS * 128 + (fo + 1) * 128],
                    ln2t[c][:, :],
                    start=(c == 0), stop=(c == NCT - 1))
            h = mp.tile([128, S], BF16, name=f"ht_{f}")
            if use_bfc:
                nc.scalar.activation(h, ps[:, :], AF.Gelu,
                                     bias=bfc_sb[:, f:f + 1], scale=1.0)
            else:
                nc.scalar.activation(h, ps[:, :], AF.Gelu, scale=1.0)
            ht.append(h)

        # Phase 7: mlp proj + residual -> out (f-outer, 8 PSUM accumulators)
        psC_cm.__exit__(None, None, None)
        psM_cm = tc.tile_pool(name="psM", bufs=8, space="PSUM")
        psM = psM_cm.__enter__()
        for g in range(1, 4):
            tl = wp.tile([128, 8 * C], BF16, name=f"mw_{g}", tag="mw", bufs=2)
            nc.sync.dma_start(out=tl, in_=mlpw[:, g * 8 * C:(g + 1) * 8 * C])
            mlp_sb[g] = tl
        accs = [psM.tile([128, 512], F32, name=f"mlps_{co}", tag="psm",
                         bufs=8) for co in range(NCT)]
        for f in range(NFT):
            g, i = f // 8, f % 8
            for co in range(NCT):
                nc.tensor.matmul(accs[co][:, :],
                                 mlp_sb[g][:, i * C + co * 128:
                                           i * C + (co + 1) * 128],
                                 ht[f][:, :],
                                 start=(f == 0), stop=(f == NFT - 1))
        for co in range(NCT):
            o = mp.tile([128, S], F32, name=f"out_{co}", tag="outt", bufs=3)
            if use_bmlp:
                nc.vector.scalar_tensor_tensor(
                    out=o, in0=accs[co][:, :], scalar=bmlp_sb[:, co:co + 1],
                    in1=x2t_sb[co], op0=ALU.add, op1=ALU.add)
            else:
                nc.vector.tensor_add(o, accs[co][:, :], x2t_sb[co])
            nc.sync.dma_start(out=out_d[co * 128:(co + 1) * 128, :], in_=o)
        psM_cm.__exit__(None, None, None)

    wt_pool.__exit__(None, None, None)
    es.close()


# =============================================================
# Host side
# =============================================================
_CACHE = {}


def _get_nc(flags):
    if flags not in _CACHE:
        _CACHE[flags] = build(flags)
    return _CACHE[flags]


def _img(w):
    """[n*128, cols] f32 -> bf16 SBUF image [128, n*cols]."""
    n = w.shape[0] // 128
    return np.ascontiguousarray(
        w.reshape(n, 128, -1).transpose(1, 0, 2).reshape(128, -1)
    ).astype(NP_BF16)


def _prep(inputs, debug=False):
    f32 = np.float32
    x = np.asarray(inputs["x"], f32)
    attn_w = np.asarray(inputs["attn_w"], f32)
    attn_b = np.asarray(inputs["attn_b"], f32)
    proj_w = np.asarray(inputs["proj_w"], f32)
    proj_b = np.asarray(inputs["proj_b"], f32)
    fc_w = np.asarray(inputs["fc_w"], f32)
    fc_b = np.asarray(inputs["fc_b"], f32)
    mlp_w = np.asarray(inputs["mlp_proj_w"], f32)
    mlp_b = np.asarray(inputs["mlp_proj_b"], f32)
    ln1w = np.asarray(inputs["ln1_w"], f32)
    ln1b = np.asarray(inputs["ln1_b"], f32)
    ln2w = np.asarray(inputs["ln2_w"], f32)
    ln2b = np.asarray(inputs["ln2_b"], f32)

    def nz(a):
        return bool(np.any(a != 0.0))

    flags = (nz(attn_b[0:C]), nz(attn_b[C:2 * C]), nz(attn_b[2 * C:3 * C]),
             nz(proj_b), nz(fc_b), nz(mlp_b),
             nz(ln1w - 1.0) or nz(ln1b), nz(ln2w - 1.0) or nz(ln2b), debug)

    def colsplit(v):
        return np.ascontiguousarray(v.reshape(-1, 128).T)

    bf = lambda a: np.ascontiguousarray(a).astype(NP_BF16)
    shared = {
        "awq": _img(attn_w[:, 0:C]),
        "awk": _img(attn_w[:, C:2 * C]),
        "awv": _img(attn_w[:, 2 * C:3 * C]),
        "pw": _img(proj_w),
        "fcw": _img(fc_w),
        "mlpw": _img(mlp_w),
        "b_q": colsplit(attn_b[0:C]), "b_k": colsplit(attn_b[C:2 * C]),
        "b_v": bf(attn_b[2 * C:3 * C].reshape(1, C)),
        "b_proj": colsplit(proj_b), "b_fc": colsplit(fc_b),
        "b_mlp": colsplit(mlp_b),
        "ln1w": colsplit(ln1w), "ln1b": colsplit(ln1b),
        "ln2w": colsplit(ln2w), "ln2b": colsplit(ln2b),
    }
    k_idx = np.arange(128)[:, None]
    q_idx = np.arange(QCH)[None, :]
    m0 = (q_idx >= k_idx).astype(NP_BF16)
    m1 = (q_idx >= k_idx + 128).astype(NP_BF16)
    one = np.ones((128, QCH), NP_BF16)
    shared["masks"] = np.concatenate(
        [m0, one, m0, one, m1, one, m1, one, m0, m0, m1, m1], axis=1)

    in_maps = []
    for i in range(CORES):
        b, s = i // 4, i % 4
        xs = np.ascontiguousarray(x[b, s * S:(s + 1) * S, :].T)  # [C, S]
        m = dict(shared)
        m["xt"] = _img(xs)
        in_maps.append(m)
    return flags, in_maps


def run_sharded(inputs, debug=False, trace=False, trace_kwargs=None):
    flags, in_maps = _prep(inputs, debug)
    nc = _get_nc(flags)
    res = bass_utils.run_bass_kernel_spmd(
        nc, in_maps, core_ids=list(range(CORES)), trace=trace,
        **(trace_kwargs or {}))
    out = np.empty((B, T, C), np.float32)
    for i in range(CORES):
        b, s = i // 4, i % 4
        out[b, s * S:(s + 1) * S, :] = res.results[i]["out"].T
    return out, res


def kernel(**inputs):
    out, _ = run_sharded(inputs, debug=False, trace=False)
    return out


# revision 16
# speedup vs baseline: 1.0638x; 1.0638x over previous
"""Trainium2 8-core kernel for a GPT-style transformer block.

Strategy (v2):
  - Token-parallel everywhere except attention: each core owns a contiguous
    512-token span (core i -> batch i//4, span i%4). LayerNorms, QKV, proj,
    MLP and residuals are purely local to the core's tokens.
  - Attention is head-parallel: ONE fused AllToAll redistributes Q^T/K^T
    (feature-major) and V (token-major) so core j holds head-pair j for all
    4096 tokens. A tiny warmup AllGather is triggered at t~0 to absorb the
    ~100us one-time collective-establishment cost during QKV compute.
  - All weights are pre-laid-out on the host into SBUF-image format
    [128, n*cols] so every load is one contiguous 2D DMA (the per-DMA issue
    cost on the sequencers is ~0.6us, so DMA count matters).
  - Softmax denominators come free via ones-columns embedded in V; the
    normalize step uses vector.reciprocal (keeps the Scalar engine's
    activation table on Exp for the whole attention phase).
  - LN2 statistics accumulate inside the proj loop; residual uses bf16 x.
"""

import sys

sys.path.insert(0, "/opt/trn_rl_repo")

import numpy as np
import ml_dtypes

import concourse.bass as bass
import concourse.mybir as mybir
import concourse.tile as tile
from concourse import bacc, bass_utils

BF16 = mybir.dt.bfloat16
F32 = mybir.dt.float32
AF = mybir.ActivationFunctionType
ALU = mybir.AluOpType
NP_BF16 = ml_dtypes.bfloat16

B, T, C, H, HS, FF = 2, 2048, 1024, 16, 64, 4096
CORES = 8
S = 512            # tokens per core
NCT = C // 128     # 8 feature tiles
NFT = FF // 128    # 32 mlp hidden tiles
NTT = S // 128     # 4 token tiles per core
QCH = 256          # query chunk
NQC = T // QCH     # 8 query chunks per batch
NKT = T // 128     # 16 key tiles per batch
EPS = 1e-5
QBLK = 65536       # per-block elems of q (and k, and v) in fused A2A


def build(flags):
    (use_bq, use_bk, use_bv, use_bproj, use_bfc, use_bmlp,
     use_ln1wb, use_ln2wb, debug) = flags

    nc = bacc.Bacc("TRN2", target_bir_lowering=False, debug=False,
                   num_devices=CORES)

    # ---------------- DRAM parameters (host-side SBUF images) ----------
    xt = nc.dram_tensor("xt", [128, NCT * S], BF16, kind="ExternalInput")
    awq = nc.dram_tensor("awq", [128, NCT * C], BF16, kind="ExternalInput")
    awk = nc.dram_tensor("awk", [128, NCT * C], BF16, kind="ExternalInput")
    awv = nc.dram_tensor("awv", [128, NCT * C], BF16, kind="ExternalInput")
    pw = nc.dram_tensor("pw", [128, NCT * C], BF16, kind="ExternalInput")
    fcw = nc.dram_tensor("fcw", [128, NCT * FF], BF16, kind="ExternalInput")
    mlpw = nc.dram_tensor("mlpw", [128, NFT * C], BF16, kind="ExternalInput")
    masks = nc.dram_tensor("masks", [128, 3 * QCH * 4], BF16,
                           kind="ExternalInput")
    b_q = nc.dram_tensor("b_q", [128, NCT], F32, kind="ExternalInput")
    b_k = nc.dram_tensor("b_k", [128, NCT], F32, kind="ExternalInput")
    b_v = nc.dram_tensor("b_v", [1, C], BF16, kind="ExternalInput")
    b_proj = nc.dram_tensor("b_proj", [128, NCT], F32, kind="ExternalInput")
    b_fc = nc.dram_tensor("b_fc", [128, NFT], F32, kind="ExternalInput")
    b_mlp = nc.dram_tensor("b_mlp", [128, NCT], F32, kind="ExternalInput")
    ln1w_d = nc.dram_tensor("ln1w", [128, NCT], F32, kind="ExternalInput")
    ln1b_d = nc.dram_tensor("ln1b", [128, NCT], F32, kind="ExternalInput")
    ln2w_d = nc.dram_tensor("ln2w", [128, NCT], F32, kind="ExternalInput")
    ln2b_d = nc.dram_tensor("ln2b", [128, NCT], F32, kind="ExternalInput")
    out_d = nc.dram_tensor("out", [C, S], F32, kind="ExternalOutput")

    with tile.TileContext(nc) as tc:
        _build_body(nc, tc, locals(), flags)
    nc.compile()
    return nc


def _build_body(nc, tc, t_, flags):
    (use_bq, use_bk, use_bv, use_bproj, use_bfc, use_bmlp,
     use_ln1wb, use_ln2wb, debug) = flags
    xt, awq, awk, awv = t_["xt"], t_["awq"], t_["awk"], t_["awv"]
    pw, fcw, mlpw, masks = t_["pw"], t_["fcw"], t_["mlpw"], t_["masks"]
    b_q, b_k, b_v, b_proj, b_fc, b_mlp = (t_["b_q"], t_["b_k"], t_["b_v"],
                                          t_["b_proj"], t_["b_fc"], t_["b_mlp"])
    ln1w_d, ln1b_d, ln2w_d, ln2b_d = (t_["ln1w_d"], t_["ln1b_d"],
                                      t_["ln2w_d"], t_["ln2b_d"])
    out_d = t_["out_d"]

    from contextlib import ExitStack
    es = ExitStack()

    consts = es.enter_context(tc.tile_pool(name="consts", bufs=1))
    dram = es.enter_context(tc.tile_pool(name="dram", bufs=1, space="DRAM"))

    # ---- collective DRAM tiles ----
    cc0_in = dram.tile([1, 128], BF16, name="cc0_in")
    cc0_out = dram.tile([CORES, 128], BF16, name="cc0_out")
    ccq_in = dram.tile([CORES, QBLK], BF16, name="ccq_in")
    ccq_out = dram.tile([CORES, QBLK], BF16, name="ccq_out")
    cck_in = dram.tile([CORES, QBLK], BF16, name="cck_in")
    cck_out = dram.tile([CORES, QBLK], BF16, name="cck_out")
    ccv_in = dram.tile([CORES, QBLK], BF16, name="ccv_in")
    ccv_out = dram.tile([CORES, QBLK], BF16, name="ccv_out")
    ccy_in = dram.tile([CORES, 128, S], BF16, name="ccy_in")
    ccy_out = dram.tile([CORES, 128, S], BF16, name="ccy_out")

    # ---- warmup collective: FIRST sync DMA + FIRST gpsimd instruction ----
    masks_sb = consts.tile([128, 3 * QCH * 4], BF16, name="masks_sb")
    nc.sync.dma_start(out=cc0_in, in_=masks[0:1, 0:128])
    nc.gpsimd.collective_compute(
        "AllGather", ALU.bypass,
        replica_groups=[list(range(CORES))],
        ins=[cc0_in[:, :].opt()],
        outs=[cc0_out[:, :].opt()])

    # ---- constants ----
    nc.sync.dma_start(out=masks_sb, in_=masks[:, :])
    mask0 = masks_sb[:, 0:4 * QCH]
    mask1 = masks_sb[:, 4 * QCH:8 * QCH]
    mask0x = masks_sb[:, 8 * QCH:10 * QCH]
    mask1x = masks_sb[:, 10 * QCH:12 * QCH]
    ones_col = consts.tile([128, 1], BF16, name="ones_col")
    nc.vector.memset(ones_col, 1.0)
    ones_row = consts.tile([1, 128], BF16, name="ones_row")
    nc.vector.memset(ones_row, 1.0)
    eps_t = consts.tile([1, 1], F32, name="eps_t")
    nc.vector.memset(eps_t, EPS)

    def load_const(name, dram_t, shape, dtype=F32):
        t = consts.tile(shape, dtype, name=name)
        nc.sync.dma_start(out=t, in_=dram_t[:, :])
        return t

    bq_sb = load_const("bq_sb", b_q, [128, NCT]) if use_bq else None
    bk_sb = load_const("bk_sb", b_k, [128, NCT]) if use_bk else None
    bv_sb = load_const("bv_sb", b_v, [1, C], BF16) if use_bv else None
    bproj_sb = load_const("bproj_sb", b_proj, [128, NCT]) if use_bproj else None
    bfc_sb = load_const("bfc_sb", b_fc, [128, NFT]) if use_bfc else None
    bmlp_sb = load_const("bmlp_sb", b_mlp, [128, NCT]) if use_bmlp else None
    ln1w_sb = load_const("ln1w_sb", ln1w_d, [128, NCT]) if use_ln1wb else None
    ln1b_sb = load_const("ln1b_sb", ln1b_d, [128, NCT]) if use_ln1wb else None
    ln2w_sb = load_const("ln2w_sb", ln2w_d, [128, NCT]) if use_ln2wb else None
    ln2b_sb = load_const("ln2b_sb", ln2b_d, [128, NCT]) if use_ln2wb else None

    def bcast(pspool, tag, src_bf, n):
        """[1, n] bf16 row -> [128, n] f32 PSUM via rank-1 matmul."""
        ps = pspool.tile([128, 512], F32, name=f"{tag}_bc", tag="ps")
        nc.tensor.matmul(ps[:, :n], ones_row[:, :], src_bf[:, :n],
                         start=True, stop=True)
        return ps

    def ln_finish(tag, pool, pspool, s_ps, q_ps):
        """From accumulated sum/sumsq [1,S] PSUM -> (r_b, sh_b) [128,S] bf16:
        normalized = src*r_b + sh_b."""
        mu = pool.tile([1, S], F32, name=f"{tag}_mu")
        nc.scalar.mul(mu, s_ps[:, :], 1.0 / C)
        msq = pool.tile([1, S], F32, name=f"{tag}_msq")
        nc.scalar.mul(msq, q_ps[:, :], 1.0 / C)
        mu2 = pool.tile([1, S], F32, name=f"{tag}_mu2")
        nc.vector.tensor_mul(mu2, mu, mu)
        var = pool.tile([1, S], F32, name=f"{tag}_var")
        nc.vector.tensor_sub(var, msq, mu2)
        lnv = pool.tile([1, S], F32, name=f"{tag}_lnv")
        nc.scalar.activation(lnv, var, AF.Ln, bias=eps_t, scale=1.0)
        rstd = pool.tile([1, S], F32, name=f"{tag}_rstd")
        nc.scalar.activation(rstd, lnv, AF.Exp, scale=-0.5)
        rstd_bf = pool.tile([1, S], BF16, name=f"{tag}_rstd_bf")
        nc.vector.tensor_copy(rstd_bf, rstd)
        nmurs = pool.tile([1, S], F32, name=f"{tag}_nmurs")
        nc.vector.tensor_mul(nmurs, mu, rstd)
        nmurs_bf = pool.tile([1, S], BF16, name=f"{tag}_nmurs_bf")
        nc.scalar.mul(nmurs_bf, nmurs, -1.0)
        r_ps = bcast(pspool, f"{tag}_r", rstd_bf, S)
        sh_ps = bcast(pspool, f"{tag}_sh", nmurs_bf, S)
        r_b = pool.tile([128, S], BF16, name=f"{tag}_r_b")
        nc.scalar.copy(r_b, r_ps[:, :S])
        sh_b = pool.tile([128, S], BF16, name=f"{tag}_sh_b")
        nc.scalar.copy(sh_b, sh_ps[:, :S])
        return r_b, sh_b

    def ln_apply(tag, pool, c, src, r_b, sh_b, w_sb, b_sb, use_wb):
        tmp = pool.tile([128, S], BF16, name=f"{tag}_tmp_{c}",
                        tag=f"{tag}_tmp", bufs=3)
        nc.vector.tensor_mul(tmp, src, r_b)
        o = pool.tile([128, S], BF16, name=f"{tag}_o_{c}")
        if use_wb:
            nc.vector.tensor_add(tmp, tmp, sh_b)
            nc.vector.tensor_scalar(
                out=o, in0=tmp,
                scalar1=w_sb[:, c:c + 1], scalar2=b_sb[:, c:c + 1],
                op0=ALU.mult, op1=ALU.add)
        else:
            nc.vector.tensor_add(o, tmp, sh_b)
        return o

    # =========================================================
    # Phase 1+2: LN1 and QKV projections
    # =========================================================
    xp = es.enter_context(tc.tile_pool(name="xt_pool", bufs=1))
    x2t_p = es.enter_context(tc.tile_pool(name="x2t_p", bufs=1))
    wt_pool = tc.tile_pool(name="wt_pool", bufs=1)
    wp = wt_pool.__enter__()

    xt_big = xp.tile([128, NCT * S], BF16, name="xt_big")
    nc.sync.dma_start(out=xt_big, in_=xt[:, :])

    # weight prefetch into a fresh SBUF region (no reuse deps): executes
    # in the first ~30us, long before the attention input loads need the
    # sync DMA channel
    pw_sb = wp.tile([128, NCT * C], BF16, name="pw_sb")
    nc.sync.dma_start(out=pw_sb, in_=pw[:, :])
    NFS = 4            # fc slabs
    FPS = NFT // NFS   # f-tiles per slab
    fc_view = fcw[:, :].rearrange("p (c f) -> p c f", f=FF)
    fw_sb = {}
    for sl in range(1):
        tl = wp.tile([128, NCT * FPS * 128], BF16, name=f"fw_{sl}",
                     tag="fw", bufs=2)
        nc.sync.dma_start(
            out=tl, in_=fc_view[:, :, sl * FPS * 128:(sl + 1) * FPS * 128])
        fw_sb[sl] = tl
    mlp_sb = {}
    for g in range(1):
        tl = wp.tile([128, 4 * C], BF16, name=f"mw_{g}", tag="mw", bufs=2)
        nc.sync.dma_start(out=tl, in_=mlpw[:, g * 4 * C:(g + 1) * 4 * C])
        mlp_sb[g] = tl

    ln1_pool = tc.tile_pool(name="ln1_pool", bufs=1)
    qkv_pool = tc.tile_pool(name="qkv_pool", bufs=1)
    psA_pool = tc.tile_pool(name="psA", bufs=6, space="PSUM")

    with ln1_pool as lp, qkv_pool as qp, psA_pool as psA:
        awq_sb = lp.tile([128, NCT * C], BF16, name="awq_sb")
        nc.sync.dma_start(out=awq_sb, in_=awq[:, :])
        awk_sb = lp.tile([128, NCT * C], BF16, name="awk_sb")
        nc.sync.dma_start(out=awk_sb, in_=awk[:, :])
        awv_sb = lp.tile([128, NCT * C], BF16, name="awv_sb")
        nc.sync.dma_start(out=awv_sb, in_=awv[:, :])

        # LN1 stats
        s_ps = psA.tile([1, 512], F32, name="ln1_sps", tag="st", bufs=2)
        q_ps = psA.tile([1, 512], F32, name="ln1_qps", tag="st", bufs=2)
        for c in range(NCT):
            src = xt_big[:, c * S:(c + 1) * S]
            sq = lp.tile([128, S], BF16, name=f"ln1_sq_{c}",
                         tag="ln1_sq", bufs=3)
            nc.vector.tensor_mul(sq, src, src)
            nc.tensor.matmul(s_ps[:, :], ones_col[:, :], src,
                             start=(c == 0), stop=(c == NCT - 1))
            nc.tensor.matmul(q_ps[:, :], ones_col[:, :], sq[:, :],
                             start=(c == 0), stop=(c == NCT - 1))
        r_b, sh_b = ln_finish("ln1", lp, psA, s_ps, q_ps)
        ln1t = [ln_apply("ln1", lp, c, xt_big[:, c * S:(c + 1) * S],
                         r_b, sh_b, ln1w_sb, ln1b_sb, use_ln1wb)
                for c in range(NCT)]

        # Q and K, feature-major [C, S], into one big output tile each;
        # each gets its own AllToAll triggered as soon as its data is out.
        for which, w_sb, bias_sb, useb, ccin, ccout in (
                ("q", awq_sb, bq_sb, use_bq, ccq_in, ccq_out),
                ("k", awk_sb, bk_sb, use_bk, cck_in, cck_out)):
            obig = qp.tile([128, NCT * S], BF16, name=f"{which}o_big")
            for hp in range(NCT):
                ps = psA.tile([128, 512], F32, name=f"{which}ps_{hp}", tag="ps")
                for c in range(NCT):
                    nc.tensor.matmul(
                        ps[:, :],
                        w_sb[:, c * C + hp * 128: c * C + (hp + 1) * 128],
                        ln1t[c][:, :],
                        start=(c == 0), stop=(c == NCT - 1))
                o = obig[:, hp * S:(hp + 1) * S]
                if useb:
                    nc.scalar.add(o, ps[:, :], bias_sb[:, hp:hp + 1])
                else:
                    nc.scalar.copy(o, ps[:, :])
            nc.sync.dma_start(
                out=ccin[:, :].rearrange("j (p s) -> p j s", p=128),
                in_=obig[:, :])
            nc.gpsimd.collective_compute(
                "AllToAll", ALU.bypass,
                replica_groups=[list(range(CORES))],
                ins=[ccin[:, :].opt()],
                outs=[ccout[:, :].opt()])

        # V, token-major [S, C]
        for tt in range(NTT):
            vo = qp.tile([128, C], BF16, name=f"vo_{tt}", tag="vo", bufs=2)
            for half in range(2):
                ps = psA.tile([128, 512], F32, name=f"vps_{tt}_{half}",
                              tag="ps")
                for c in range(NCT):
                    nc.tensor.matmul(
                        ps[:, :],
                        ln1t[c][:, tt * 128:(tt + 1) * 128],
                        awv_sb[:, c * C + half * 512: c * C + (half + 1) * 512],
                        start=(c == 0), stop=(c == NCT - 1 and not use_bv))
                if use_bv:
                    nc.tensor.matmul(
                        ps[:, :], ones_row[:, :],
                        bv_sb[:, half * 512:(half + 1) * 512],
                        start=False, stop=True)
                nc.scalar.copy(vo[:, half * 512:(half + 1) * 512], ps[:, :])
            nc.sync.dma_start(
                out=ccv_in[:, tt * (128 * 128):
                           (tt + 1) * (128 * 128)].rearrange(
                    "j (t f) -> t j f", f=128),
                in_=vo[:, :])

        nc.gpsimd.collective_compute(
            "AllToAll", ALU.bypass,
            replica_groups=[list(range(CORES))],
            ins=[ccv_in[:, :].opt()],
            outs=[ccv_out[:, :].opt()])

    # =========================================================
    # Phase 3: attention (my 2 heads, all tokens)
    # =========================================================
    att_pool = tc.tile_pool(name="att_pool", bufs=1)
    psB_pool = tc.tile_pool(name="psB", bufs=2, space="PSUM")
    with att_pool as ap, psB_pool as psB:
        qtb, ktb, vt = [], [], []
        for b in range(B):
            qt_t = ap.tile([128, T], BF16, name=f"qtb_{b}")
            nc.sync.dma_start(
                out=qt_t,
                in_=ccq_out[4 * b:4 * b + 4, :].rearrange(
                    "r (p s) -> p r s", p=128))
            kt_t = ap.tile([128, T], BF16, name=f"ktb_{b}")
            nc.scalar.dma_start(
                out=kt_t,
                in_=cck_out[4 * b:4 * b + 4, :].rearrange(
                    "r (p s) -> p r s", p=128))
            qtb.append(qt_t)
            ktb.append(kt_t)
        for b in range(B):
            for kt in range(NKT):
                # per head: col 0 = ones (denominator), 1:64 zero, 64:128 = V
                v3 = ap.tile([128, 2, 128], BF16, name=f"vt_{b}_{kt}")
                r = 4 * b + kt // 4
                ro = (kt % 4) * 128
                nc.gpsimd.memset(v3[:, :, 0:1], 1.0)
                nc.gpsimd.memset(v3[:, :, 1:64], 0.0)
                nc.sync.dma_start(
                    out=v3[:, :, 64:128],
                    in_=ccv_out[r, ro * 128:(ro + 128) * 128].rearrange(
                        "(t h f) -> t h f", h=2, f=64))
                vt.append(v3)

        W2 = 2 * QCH
        for b in range(B):
            for p in reversed(range(NQC // 2)):
                qc = 2 * p
                qs = qc * QCH
                nsh = 2 * (qc + 1)          # shared key tiles
                y_A = psB.tile([128, W2], F32, name=f"yA_{b}_{p}", tag="ya",
                               bufs=4)
                y_B = psB.tile([128, W2], F32, name=f"yB_{b}_{p}", tag="ya",
                               bufs=4)
                ytA = ap.tile([128, W2], BF16, name=f"ytbA_{b}_{p}",
                              tag="ytA", bufs=3)
                ytB = ap.tile([128, W2], BF16, name=f"ytbB_{b}_{p}",
                              tag="ytB", bufs=3)
                # phase 1: all scores + exp (+mask) for this pair -- lets
                # score work run while the V AllToAll is still in flight
                es_AB = []
                for kt in range(nsh + 2):
                    shared = kt < nsh
                    ncols = W2 if shared else QCH
                    s_AB = psB.tile([128, 2 * W2], F32, name=f"s_{b}_{p}_{kt}",
                                    tag="ps2", bufs=2)
                    cols = slice(0, W2) if shared else slice(QCH, W2)
                    nc.tensor.matmul(s_AB[:, 0:ncols],
                                     ktb[b][0:64, kt * 128:(kt + 1) * 128],
                                     qtb[b][0:64, qs + cols.start:qs + W2],
                                     start=True, stop=True)
                    nc.tensor.matmul(s_AB[:, W2:W2 + ncols],
                                     ktb[b][64:128, kt * 128:(kt + 1) * 128],
                                     qtb[b][64:128, qs + cols.start:qs + W2],
                                     start=True, stop=True)
                    e_AB = ap.tile([128, 2 * W2], BF16, name=f"e_{b}_{p}_{kt}",
                                   tag="eAB", bufs=16)
                    if shared:
                        nc.scalar.activation(e_AB, s_AB[:, :], AF.Exp,
                                             scale=1.0 / np.sqrt(HS))
                        if kt == qc * 2:
                            nc.vector.tensor_mul(e_AB, e_AB, mask0)
                        elif kt == qc * 2 + 1:
                            nc.vector.tensor_mul(e_AB, e_AB, mask1)
                    else:
                        e3 = e_AB.rearrange("p (h q) -> p h q", h=2)
                        s3 = s_AB.rearrange("p (h q) -> p h q", h=2)
                        nc.scalar.activation(e3[:, :, 0:QCH], s3[:, :, 0:QCH],
                                             AF.Exp, scale=1.0 / np.sqrt(HS))
                        mx = mask0x if kt == nsh else mask1x
                        nc.vector.tensor_mul(
                            e_AB.rearrange("p (h q) -> p h q", h=2)[:, :, 0:QCH],
                            e_AB.rearrange("p (h q) -> p h q", h=2)[:, :, 0:QCH],
                            mx.rearrange("p (h q) -> p h q", h=2))
                    es_AB.append(e_AB)
                # phase 2: all AV accumulations
                for kt in range(nsh + 2):
                    shared = kt < nsh
                    cols = slice(0, W2) if shared else slice(QCH, W2)
                    ncols = W2 if shared else QCH
                    e_AB = es_AB[kt]
                    v3 = vt[b * NKT + kt]
                    nc.tensor.matmul(y_A[:, cols], v3[:, 0, :],
                                     e_AB[:, 0:ncols],
                                     start=(kt == 0), stop=(kt == nsh + 1),
                                     skip_group_check=True)
                    nc.tensor.matmul(y_B[:, cols], v3[:, 1, :],
                                     e_AB[:, W2:W2 + ncols],
                                     start=(kt == 0),
                                     stop=(kt == nsh + 1),
                                     skip_group_check=True)
                # normalize: den = row 0 of PSUM acc; recip on Vector engine
                for y_ps, ytb, hn in ((y_A, ytA, "A"), (y_B, ytB, "B")):
                    nc.vector.tensor_copy(ytb[64:128, :], y_ps[64:128, :])
                    rec = ap.tile([1, W2], F32, name=f"rec_{b}_{p}{hn}",
                                  tag="rec", bufs=4)
                    nc.vector.reciprocal_approx_fast(rec, y_ps[0:1, :])
                    rec_bf = ap.tile([1, W2], BF16, name=f"recbf_{b}_{p}{hn}",
                                     tag="recbf", bufs=4)
                    nc.vector.tensor_copy(rec_bf, rec)
                    den = ap.tile([128, W2], BF16, name=f"den_{b}_{p}{hn}",
                                  tag="den", bufs=4)
                    nc.gpsimd.partition_broadcast(den, rec_bf)
                    nc.vector.tensor_mul(ytb[64:128, :],
                                         ytb[64:128, :], den[64:128, :])
                j = 4 * b + p
                nc.sync.dma_start(out=ccy_in[j, 0:64, :], in_=ytA[64:128, :])
                nc.sync.dma_start(out=ccy_in[j, 64:128, :], in_=ytB[64:128, :])

        a2a_y = nc.gpsimd.collective_compute(
            "AllToAll", ALU.bypass,
            replica_groups=[list(range(CORES))],
            ins=[ccy_in[:, :, :].opt()],
            outs=[ccy_out[:, :, :].opt()])

    # =========================================================
    # Phase 4: proj + residual (+ LN2 stats inline)
    # =========================================================
    mlp_pool = tc.tile_pool(name="mlp_pool", bufs=1)
    psC_cm = tc.tile_pool(name="psC", bufs=6, space="PSUM")
    psC = psC_cm.__enter__()
    with mlp_pool as mp:
        yta = mp.tile([128, NCT * S], BF16, name="yta_big")
        nc.sync.dma_start(
            out=yta, in_=ccy_out[:, :, :].rearrange("j p s -> p j s"))
        s2_ps = psC.tile([1, 512], F32, name="ln2_sps", tag="st", bufs=2)
        q2_ps = psC.tile([1, 512], F32, name="ln2_qps", tag="st", bufs=2)
        x2t_sb, x2bf_sb = [], []
        for co in range(NCT):
            ps = psC.tile([128, 512], F32, name=f"prps_{co}", tag="ps")
            for ci in range(NCT):
                nc.tensor.matmul(ps[:, :],
                                 pw_sb[:, ci * C + co * 128:
                                       ci * C + (co + 1) * 128],
                                 yta[:, ci * S:(ci + 1) * S],
                                 start=(ci == 0), stop=(ci == NCT - 1))
            x2 = x2t_p.tile([128, S], F32, name=f"x2t_{co}")
            if use_bproj:
                nc.vector.scalar_tensor_tensor(
                    out=x2, in0=ps[:, :], scalar=bproj_sb[:, co:co + 1],
                    in1=xt_big[:, co * S:(co + 1) * S],
                    op0=ALU.add, op1=ALU.add)
            else:
                nc.vector.tensor_add(x2, ps[:, :],
                                     xt_big[:, co * S:(co + 1) * S])
            x2b = mp.tile([128, S], BF16, name=f"x2bf_{co}")
            nc.vector.tensor_copy(x2b, x2)
            sq2 = mp.tile([128, S], BF16, name=f"ln2_sq_{co}",
                          tag="ln2_sq", bufs=3)
            nc.vector.tensor_mul(sq2, x2b, x2b)
            nc.tensor.matmul(s2_ps[:, :], ones_col[:, :], x2b[:, :],
                             start=(co == 0), stop=(co == NCT - 1))
            nc.tensor.matmul(q2_ps[:, :], ones_col[:, :], sq2[:, :],
                             start=(co == 0), stop=(co == NCT - 1))
            x2t_sb.append(x2)
            x2bf_sb.append(x2b)

        # Phase 5: LN2 finish + apply
        r2_b, sh2_b = ln_finish("ln2", mp, psC, s2_ps, q2_ps)
        ln2t = [ln_apply("ln2", mp, c, x2bf_sb[c], r2_b, sh2_b,
                         ln2w_sb, ln2b_sb, use_ln2wb)
                for c in range(NCT)]

        # Phase 6: fc + GELU  (fc weights in 4 slabs, ring of 2)
        for sl in range(1, NFS):
            tl = wp.tile([128, NCT * FPS * 128], BF16, name=f"fw_{sl}",
                         tag="fw", bufs=2)
            nc.sync.dma_start(
                out=tl,
                in_=fc_view[:, :, sl * FPS * 128:(sl + 1) * FPS * 128])
            fw_sb[sl] = tl
        ht = []
        for f in range(NFT):
            sl, fo = f // FPS, f % FPS
            ps = psC.tile([128, 512], F32, name=f"fcps_{f}", tag="ps")
            for c in range(NCT):
                nc.tensor.matmul(
                    ps[:, :],
                    fw_sb[sl][:, c * FPS * 128 + fo * 128:
                              c * FPS * 128 + (fo + 1) * 128],
                    ln2t[c][:, :],
                    start=(c == 0), stop=(c == NCT - 1))
            h = mp.tile([128, S], BF16, name=f"ht_{f}")
            if use_bfc:
                nc.scalar.activation(h, ps[:, :], AF.Gelu,
                                     bias=bfc_sb[:, f:f + 1], scale=1.0)
            else:
                nc.scalar.activation(h, ps[:, :], AF.Gelu, scale=1.0)
            ht.append(h)

        # Phase 7: mlp proj + residual -> out (f-outer, 8 PSUM accumulators)
        psC_cm.__exit__(None, None, None)
        psM_cm = tc.tile_pool(name="psM", bufs=8, space="PSUM")
        psM = psM_cm.__enter__()
        for g in range(1, 8):
            tl = wp.tile([128, 4 * C], BF16, name=f"mw_{g}", tag="mw", bufs=2)
            nc.sync.dma_start(out=tl, in_=mlpw[:, g * 4 * C:(g + 1) * 4 * C])
            mlp_sb[g] = tl
        accs = [psM.tile([128, 512], F32, name=f"mlps_{co}", tag="psm",
                         bufs=8) for co in range(NCT)]
        for f in range(NFT):
            g, i = f // 4, f % 4
            for co in range(NCT):
                nc.tensor.matmul(accs[co][:, :],
                                 mlp_sb[g][:, i * C + co * 128:
                                           i * C + (co + 1) * 128],
                                 ht[f][:, :],
                                 start=(f == 0), stop=(f == NFT - 1))
        for co in range(NCT):
            o = mp.tile([128, S], F32, name=f"out_{co}", tag="outt", bufs=3)
            if use_bmlp:
                nc.vector.scalar_tensor_tensor(
                    out=o, in0=accs[co][:, :], scalar=bmlp_sb[:, co:co + 1],
                    in1=x2t_sb[co], op0=ALU.add, op1=ALU.add)
            else:
                nc.vector.tensor_add(o, accs[co][:, :], x2t_sb[co])
            nc.sync.dma_start(out=out_d[co * 128:(co + 1) * 128, :], in_=o)
        psM_cm.__exit__(None, None, None)

    wt_pool.__exit__(None, None, None)
    es.close()


# =============================================================
# Host side
# =============================================================
_CACHE = {}


def _get_nc(flags):
    if flags not in _CACHE:
        _CACHE[flags] = build(flags)
    return _CACHE[flags]


def _img(w):
    """[n*128, cols] f32 -> bf16 SBUF image [128, n*cols]."""
    n = w.shape[0] // 128
    return np.ascontiguousarray(
        w.reshape(n, 128, -1).transpose(1, 0, 2).reshape(128, -1)
    ).astype(NP_BF16)


def _prep(inputs, debug=False):
    f32 = np.float32
    x = np.asarray(inputs["x"], f32)
    attn_w = np.asarray(inputs["attn_w"], f32)
    attn_b = np.asarray(inputs["attn_b"], f32)
    proj_w = np.asarray(inputs["proj_w"], f32)
    proj_b = np.asarray(inputs["proj_b"], f32)
    fc_w = np.asarray(inputs["fc_w"], f32)
    fc_b = np.asarray(inputs["fc_b"], f32)
    mlp_w = np.asarray(inputs["mlp_proj_w"], f32)
    mlp_b = np.asarray(inputs["mlp_proj_b"], f32)
    ln1w = np.asarray(inputs["ln1_w"], f32)
    ln1b = np.asarray(inputs["ln1_b"], f32)
    ln2w = np.asarray(inputs["ln2_w"], f32)
    ln2b = np.asarray(inputs["ln2_b"], f32)

    def nz(a):
        return bool(np.any(a != 0.0))

    flags = (nz(attn_b[0:C]), nz(attn_b[C:2 * C]), nz(attn_b[2 * C:3 * C]),
             nz(proj_b), nz(fc_b), nz(mlp_b),
             nz(ln1w - 1.0) or nz(ln1b), nz(ln2w - 1.0) or nz(ln2b), debug)

    def colsplit(v):
        return np.ascontiguousarray(v.reshape(-1, 128).T)

    bf = lambda a: np.ascontiguousarray(a).astype(NP_BF16)
    shared = {
        "awq": _img(attn_w[:, 0:C]),
        "awk": _img(attn_w[:, C:2 * C]),
        "awv": _img(attn_w[:, 2 * C:3 * C]),
        "pw": _img(proj_w),
        "fcw": _img(fc_w),
        "mlpw": _img(mlp_w),
        "b_q": colsplit(attn_b[0:C]), "b_k": colsplit(attn_b[C:2 * C]),
        "b_v": bf(attn_b[2 * C:3 * C].reshape(1, C)),
        "b_proj": colsplit(proj_b), "b_fc": colsplit(fc_b),
        "b_mlp": colsplit(mlp_b),
        "ln1w": colsplit(ln1w), "ln1b": colsplit(ln1b),
        "ln2w": colsplit(ln2w), "ln2b": colsplit(ln2b),
    }
    k_idx = np.arange(128)[:, None]
    q_idx = np.arange(QCH)[None, :]
    m0 = (q_idx >= k_idx).astype(NP_BF16)
    m1 = (q_idx >= k_idx + 128).astype(NP_BF16)
    one = np.ones((128, QCH), NP_BF16)
    shared["masks"] = np.concatenate(
        [m0, one, m0, one, m1, one, m1, one, m0, m0, m1, m1], axis=1)

    in_maps = []
    for i in range(CORES):
        b, s = i // 4, i % 4
        xs = np.ascontiguousarray(x[b, s * S:(s + 1) * S, :].T)  # [C, S]
        m = dict(shared)
        m["xt"] = _img(xs)
        in_maps.append(m)
    return flags, in_maps


def run_sharded(inputs, debug=False, trace=False, trace_kwargs=None):
    flags, in_maps = _prep(inputs, debug)
    nc = _get_nc(flags)
    res = bass_utils.run_bass_kernel_spmd(
        nc, in_maps, core_ids=list(range(CORES)), trace=trace,
        **(trace_kwargs or {}))
    out = np.empty((B, T, C), np.float32)
    for i in range(CORES):
        b, s = i // 4, i % 4
        out[b, s * S:(s + 1) * S, :] = res.results[i]["out"].T
    return out, res


def kernel(**inputs):
    out, _ = run_sharded(inputs, debug=False, trace=False)
    return out
